# revision 7
# baseline (speedup 1.0000x reference)
"""Trainium2 Bass kernel for GQA attention with QK-RMSNorm + YaRN RoPE.

Sharding: 8 cores = 2 (batch) x 4 (KV group). Each core owns one batch
element and one KV group (4 query heads + 1 KV head). The output
projection is split along its contraction dim, so each core produces a
partial (T, D) output; the host sums the 4 group-partials per batch.

v2 highlights (vs v1 baseline, 318 us):
- All 128x128 head transposes moved off the PE onto the DMA xbar
  (dma_start(transpose=True) on the Activation HWDGE queue).
- Dual DMA queues: x tiles + outputs on Sync, weights/tables/transposes
  on Activation; everything prefetched up-front.
- Phase A projections run k-innermost over j-pairs so chunk-0 compute
  paces with DMA arrival; rope is 4 batched DVE ops (broadcast APs) +
  per-head norm-scale instead of ~20 per-head ops.
- Phase B runs a depth-3 software pipeline: scores (+exp on ACT) issue
  ahead of PV/denominator matmuls so the PE never waits on Exp; the
  out-projection (phase C) groups are interleaved as PE gap fillers.
"""

import math
import sys

import numpy as np

if "/opt/trn_rl_repo" not in sys.path:
    sys.path.insert(0, "/opt/trn_rl_repo")

import ml_dtypes

import concourse.bass as bass
import concourse.tile as tile
from concourse import bacc, mybir
from concourse.bass import broadcast_tensor_aps
from concourse.bass_utils import run_bass_kernel_spmd

# Problem constants (hardcoded; kernel.py must be self-contained).
B, T, D = 2, 2048, 2048
DH, NQ, NKV = 128, 16, 4
QPG = NQ // NKV  # 4 query heads per KV group
ROPE_BASE = 10000.0
YARN_SCALE = 2.0
ORIG_MAX_LEN = 4096
BETA_FAST, BETA_SLOW = 32.0, 1.0
EPS = 1.1920929e-07
MSCALE = 0.1 * math.log(YARN_SCALE) + 1.0
ATTN_SCALE = 1.0 / (MSCALE * math.sqrt(DH))

NC = 8  # cores
TC = 512  # tokens per chunk
NCHUNK = T // TC  # 4
NST = T // 128  # 16 s-tiles (128 tokens each)
NH = QPG + 1  # 4 q heads + 1 k head handled together in phase A

F32 = mybir.dt.float32
BF16 = mybir.dt.bfloat16
NPBF16 = ml_dtypes.bfloat16


def _yarn_inv_freq():
    inv_freq = 1.0 / ROPE_BASE ** (np.arange(0, DH, 2, dtype=np.float32) / DH)
    wavelengths = 2.0 * math.pi / inv_freq
    low_w = ORIG_MAX_LEN / BETA_SLOW
    high_w = ORIG_MAX_LEN / BETA_FAST
    gamma = np.clip((low_w - wavelengths) / (low_w - high_w), 0.0, 1.0)
    return (gamma * inv_freq + (1.0 - gamma) * inv_freq / YARN_SCALE).astype(np.float32)


def _rope_tables():
    t = np.arange(T, dtype=np.float32)
    freqs = np.outer(t, _yarn_inv_freq())  # (T, 64)
    emb = np.concatenate([freqs, freqs], axis=-1)  # (T, 128)
    cos = np.cos(emb).astype(np.float32)
    sin = np.sin(emb).astype(np.float32)
    # Signed sin table: rope term2[:, :64] = q[:, 64:] * (-sin[:, :64]),
    # term2[:, 64:] = q[:, :64] * (+sin[:, 64:]).
    sinw = sin.copy()
    sinw[:, : DH // 2] *= -1.0
    return cos, sinw


def build_graph():
    nc = bacc.Bacc("TRN2", target_bir_lowering=False, debug=False, num_devices=NC)

    xT_d = nc.dram_tensor("xT", [D, T], BF16, kind="ExternalInput").ap()
    wq_d = nc.dram_tensor("wq", [D, QPG * DH], BF16, kind="ExternalInput").ap()
    wkv_d = nc.dram_tensor("wkv", [D, 2 * DH], BF16, kind="ExternalInput").ap()
    wout_d = nc.dram_tensor("wout", [QPG * DH, D], BF16, kind="ExternalInput").ap()
    cos_d = nc.dram_tensor("cosw", [T, DH], BF16, kind="ExternalInput").ap()
    sin_d = nc.dram_tensor("sinw", [T, DH], BF16, kind="ExternalInput").ap()
    mask_d = nc.dram_tensor("mask", [128, 128], BF16, kind="ExternalInput").ap()
    out_d = nc.dram_tensor("out", [T, D], BF16, kind="ExternalOutput").ap()

    with tile.TileContext(nc) as tc:
        with tc.tile_pool(name="persist", bufs=1) as pp:
            # Head-major transposed activations: [dh, t] per head, bf16.
            qT = pp.tile([128, QPG, T], BF16)
            kT = pp.tile([128, T], BF16)
            v_tok = pp.tile([128, NST, DH], BF16)  # token-major V
            oT = pp.tile([128, QPG, T], BF16)
            mask_sb = pp.tile([128, 128], BF16)
            ones_mat = pp.tile([128, 128], BF16)
            eps_col = pp.tile([128, 1], F32)
            wout_t = pp.tile([128, QPG, NCHUNK, 512], BF16)
            nc.sync.dma_start(mask_sb[:], mask_d[:])
            nc.vector.memset(ones_mat[:], 1.0)
            nc.vector.memset(eps_col[:], EPS)

            # ---------------- Phase A: projections + norm + rope ----------
            with (
                tc.tile_pool(name="wA", bufs=1) as wA,
                tc.tile_pool(name="xt", bufs=56) as xtp,
                tc.tile_pool(name="ropetab", bufs=8) as rtp,
                tc.tile_pool(name="psA_q", bufs=4, space="PSUM") as psq,
                tc.tile_pool(name="psA_kv", bufs=4, space="PSUM") as pskv,
                tc.tile_pool(name="qk", bufs=8) as qkp,
                tc.tile_pool(name="sq", bufs=4) as sqp,
                tc.tile_pool(name="small", bufs=12) as smp,
                tc.tile_pool(name="rope", bufs=2) as rp,
                tc.tile_pool(name="hat", bufs=6) as hp,
            ):
                wq_t = wA.tile([128, D // 128, QPG * DH], BF16)
                wkv_t = wA.tile([128, D // 128, 2 * DH], BF16)

                # x tiles on the Sync HWDGE queue (all chunks up front).
                xt = []
                for c in range(NCHUNK):
                    row = []
                    for k in range(D // 128):
                        xk = xtp.tile([128, TC], BF16, name=f"xt_{c}_{k}", tag="xt")
                        nc.sync.dma_start(
                            xk[:],
                            xT_d[128 * k : 128 * (k + 1), TC * c : TC * (c + 1)],
                        )
                        row.append(xk)
                    xt.append(row)

                # Weights + rope tables on the Activation HWDGE queue.
                for k in range(D // 128):
                    nc.scalar.dma_start(
                        wq_t[:, k, :], wq_d[128 * k : 128 * (k + 1), :]
                    )
                    nc.scalar.dma_start(
                        wkv_t[:, k, :], wkv_d[128 * k : 128 * (k + 1), :]
                    )
                nc.scalar.dma_start(
                    wout_t[:],
                    wout_d.rearrange("(h p) (c n) -> p h c n", p=128, n=512),
                )
                cos_ts, sin_ts = [], []
                for c in range(NCHUNK):
                    cos_t = rtp.tile([128, 4, DH], BF16, name=f"cos_{c}", tag="cos")
                    sin_t = rtp.tile([128, 4, DH], BF16, name=f"sin_{c}", tag="sin")
                    nc.scalar.dma_start(
                        cos_t[:],
                        cos_d[TC * c : TC * (c + 1), :].rearrange(
                            "(j p) d -> p j d", p=128
                        ),
                    )
                    nc.scalar.dma_start(
                        sin_t[:],
                        sin_d[TC * c : TC * (c + 1), :].rearrange(
                            "(j p) d -> p j d", p=128
                        ),
                    )
                    cos_ts.append(cos_t)
                    sin_ts.append(sin_t)

                for c in range(NCHUNK):
                    for half in range(2):
                        ps_qs = [
                            psq.tile([128, 512], F32, name=f"psq_{c}_{half}_{i}", tag="psq")
                            for i in range(2)
                        ]
                        ps_ks = [
                            pskv.tile([128, 256], F32, name=f"psk_{c}_{half}_{i}", tag="psk")
                            for i in range(2)
                        ]
                        for k in range(D // 128):
                            for jj in range(2):
                                j = 2 * half + jj
                                x_sl = xt[c][k][:, 128 * j : 128 * (j + 1)]
                                nc.tensor.matmul(
                                    ps_qs[jj][:],
                                    x_sl,
                                    wq_t[:, k, :],
                                    start=(k == 0),
                                    stop=(k == D // 128 - 1),
                                )
                                nc.tensor.matmul(
                                    ps_ks[jj][:],
                                    x_sl,
                                    wkv_t[:, k, :],
                                    start=(k == 0),
                                    stop=(k == D // 128 - 1),
                                )
                        for jj in range(2):
                            j = 2 * half + jj
                            st = 4 * c + j
                            # Token-major q||k tile: heads 0..3 are q, 4 is k.
                            qk = qkp.tile([128, NH * DH], BF16, name=f"qk_{st}", tag="qk")
                            nc.vector.tensor_copy(qk[:, 0:512], ps_qs[jj][:])
                            nc.vector.tensor_copy(qk[:, 512:640], ps_ks[jj][:, 0:128])
                            nc.vector.tensor_copy(v_tok[:, st, :], ps_ks[jj][:, 128:256])
                            # RMSNorm stats on ACT: Square dumps q2, accum_out
                            # sums over dh -> ssq col per head.
                            ssq = smp.tile([128, NH], F32, name=f"ssq_{st}", tag="ssq")
                            q2 = sqp.tile([128, 128], F32, name=f"q2_{st}", tag="q2")
                            for h in range(NH):
                                nc.scalar.activation(
                                    q2[:], qk[:, 128 * h : 128 * (h + 1)],
                                    mybir.ActivationFunctionType.Square,
                                    accum_out=ssq[:, h : h + 1],
                                )
                            # rsqrt(ms + eps) = exp(-0.5 * ln(ssq/128 + eps))
                            lnv = smp.tile([128, NH], F32, name=f"ln_{st}", tag="lnv")
                            scv = smp.tile([128, NH], F32, name=f"sc_{st}", tag="scv")
                            nc.scalar.activation(
                                lnv[:], ssq[:], mybir.ActivationFunctionType.Ln,
                                bias=eps_col[:], scale=1.0 / DH,
                            )
                            nc.scalar.activation(
                                scv[:], lnv[:], mybir.ActivationFunctionType.Exp,
                                bias=0.0, scale=-0.5,
                            )
                            # Normalize, then rope via batched DVE ops.
                            qks = rp.tile([128, NH * DH], BF16, name=f"qks_{st}", tag="qks")
                            for h in range(NH):
                                nc.vector.tensor_scalar_mul(
                                    qks[:, 128 * h : 128 * (h + 1)],
                                    qk[:, 128 * h : 128 * (h + 1)],
                                    scv[:, h : h + 1],
                                )
                            qhat = hp.tile([128, NH * DH], BF16, name=f"qh_{st}", tag="qh")
                            t1 = rp.tile([128, NH * DH], BF16, name=f"t1_{st}", tag="t1")
                            t2 = rp.tile([128, NH * DH], BF16, name=f"t2_{st}", tag="t2")
                            # t1 = qks * cos (cos broadcast across the 5 heads)
                            a0 = qks[:].rearrange("p (h d) -> p h d", h=NH)
                            a1 = cos_ts[c][:, j, :].rearrange("p (o d) -> p o d", o=1)
                            b0, b1 = broadcast_tensor_aps(a0, a1)
                            t1r = t1[:].rearrange("p (h d) -> p h d", h=NH)
                            nc.vector.tensor_mul(t1r, b0, b1)
                            # t2 halves: lower-out = upper-in * (-sin_low),
                            # upper-out = lower-in * (+sin_high)
                            q4 = qks[:].rearrange("p (h t d) -> p h t d", h=NH, t=2)
                            t24 = t2[:].rearrange("p (h t d) -> p h t d", h=NH, t=2)
                            s_lo = sin_ts[c][:, j, 0:64].rearrange("p (o u d) -> p o u d", o=1, u=1)
                            s_hi = sin_ts[c][:, j, 64:128].rearrange("p (o u d) -> p o u d", o=1, u=1)
                            c0, c1 = broadcast_tensor_aps(q4[:, :, 1:2, :], s_lo)
                            nc.vector.tensor_mul(t24[:, :, 0:1, :], c0, c1)
                            c2, c3 = broadcast_tensor_aps(q4[:, :, 0:1, :], s_hi)
                            nc.vector.tensor_mul(t24[:, :, 1:2, :], c2, c3)
                            nc.vector.tensor_add(qhat[:], t1[:], t2[:])
                            # Head transposes via DMA xbar (Activation queue).
                            for h in range(NH):
                                if h < QPG:
                                    dst = qT[:, h, 128 * st : 128 * (st + 1)]
                                else:
                                    dst = kT[:, 128 * st : 128 * (st + 1)]
                                nc.scalar.dma_start(
                                    dst, qhat[:, 128 * h : 128 * (h + 1)],
                                    transpose=True,
                                )

            # ---------------- Phase B + C: attention + out projection -----
            with (
                tc.tile_pool(name="psB_s", bufs=4, space="PSUM") as pss,
                tc.tile_pool(name="psB_o", bufs=2, space="PSUM") as pso,
                tc.tile_pool(name="psB_d", bufs=2, space="PSUM") as psd,
                tc.tile_pool(name="ebuf", bufs=8) as ep,
                tc.tile_pool(name="bcs", bufs=3) as bcp,
                tc.tile_pool(name="osb", bufs=4) as osp,
            ):
                units = []
                for j in range(NCHUNK):
                    for h in range(QPG):
                        for st in range(4 * (j + 1)):
                            units.append((j, h, st))

                # Plan: B units with C(j-1) groups interleaved as fillers;
                # C(3) drains at the end.
                plan = []
                base = 0
                for j in range(NCHUNK):
                    n_u = QPG * 4 * (j + 1)
                    cfills = (
                        [("c", j - 1, jj, dc) for jj in range(4) for dc in range(4)]
                        if j >= 1
                        else []
                    )
                    ci = 0
                    for k in range(n_u):
                        plan.append(("u", base + k))
                        if ci < len(cfills) and (k % (j + 1)) == j:
                            plan.append(cfills[ci])
                            ci += 1
                    for item in cfills[ci:]:
                        plan.append(item)
                    base += n_u
                for jj in range(4):
                    for dc in range(4):
                        plan.append(("c", NCHUNK - 1, jj, dc))

                def emit_scores(u):
                    j, h, st = u
                    d0 = max(0, 128 * (st - 4 * j))
                    ps_s = pss.tile([128, 512], F32, name=f"pss_{j}_{h}_{st}", tag="pss")
                    nc.tensor.matmul(
                        ps_s[:, d0:512],
                        kT[:, 128 * st : 128 * (st + 1)],
                        qT[:, h, TC * j + d0 : TC * (j + 1)],
                        start=True,
                        stop=True,
                    )
                    E = ep.tile([128, 512], BF16, name=f"E_{j}_{h}_{st}", tag="E")
                    nc.scalar.activation(
                        E[:, d0:512], ps_s[:, d0:512],
                        mybir.ActivationFunctionType.Exp,
                        bias=0.0, scale=ATTN_SCALE,
                    )
                    if st >= 4 * j:  # diagonal block is triangular
                        nc.vector.tensor_mul(
                            E[:, d0 : d0 + 128], E[:, d0 : d0 + 128], mask_sb[:]
                        )
                    return (E, d0)

                cur = {}

                def emit_pv(u, ed):
                    j, h, st = u
                    E, d0 = ed
                    S = 4 * (j + 1)
                    if st == 0:
                        cur["o"] = pso.tile([128, 512], F32, name=f"pso_{j}_{h}", tag="pso")
                        cur["d"] = psd.tile([128, 512], F32, name=f"psd_{j}_{h}", tag="psd")
                    nc.tensor.matmul(
                        cur["o"][:, d0:512],
                        v_tok[:, st, :],
                        E[:, d0:512],
                        start=(st == 0),
                        stop=(st == S - 1),
                    )
                    nc.tensor.matmul(
                        cur["d"][:, d0:512],
                        ones_mat[:],
                        E[:, d0:512],
                        start=(st == 0),
                        stop=(st == S - 1),
                    )
                    if st == S - 1:
                        bc = bcp.tile([128, 512], F32, name=f"bc_{j}_{h}", tag="bc")
                        nc.vector.reciprocal_approx_fast(bc[:], cur["d"][:])
                        nc.vector.tensor_mul(
                            oT[:, h, TC * j : TC * (j + 1)], cur["o"][:], bc[:]
                        )

                def emit_cgroup(j, jj, dc):
                    t0 = TC * j + 128 * jj
                    ps_c = pss.tile([128, 512], F32, name=f"psc_{j}_{jj}_{dc}", tag="pss")
                    for h in range(QPG):
                        nc.tensor.matmul(
                            ps_c[:],
                            oT[:, h, t0 : t0 + 128],
                            wout_t[:, h, dc, :],
                            start=(h == 0),
                            stop=(h == QPG - 1),
                        )
                    o_sb = osp.tile([128, 512], BF16, name=f"o_{j}_{dc}_{jj}", tag="o")
                    nc.vector.tensor_copy(o_sb[:], ps_c[:])
                    nc.sync.dma_start(
                        out_d[t0 : t0 + 128, 512 * dc : 512 * (dc + 1)], o_sb[:]
                    )

                sp = 0  # scores-emitted pointer (runs ahead of PV)
                einfo = {}
                for item in plan:
                    if item[0] == "u":
                        i = item[1]
                        while sp < min(i + 3, len(units)):
                            einfo[sp] = emit_scores(units[sp])
                            sp += 1
                        emit_pv(units[i], einfo.pop(i))
                    else:
                        emit_cgroup(item[1], item[2], item[3])

    nc.compile()
    return nc


def shard_inputs(x, Wq, Wkv, Wout, q_norm_w, k_norm_w, inv_freq):
    """Build per-core input maps. Weights/x are pre-cast to bf16 on host
    (compute dtype), halving their HBM traffic."""
    cos, sinw = _rope_tables()
    qw = np.asarray(q_norm_w, np.float32)
    kw = np.asarray(k_norm_w, np.float32)
    assert np.allclose(qw, 1.0) and np.allclose(kw, 1.0), "non-unit norm weights"

    mask = np.triu(np.ones((128, 128), np.float32)).astype(NPBF16)
    Wq4 = np.asarray(Wq, np.float32).reshape(D, QPG, NKV, DH)
    Wkv2 = np.asarray(Wkv, np.float32)
    Wout4 = np.asarray(Wout, np.float32).reshape(QPG, NKV, DH, D)
    x = np.asarray(x, np.float32)

    in_maps = []
    for core in range(NC):
        b, g = divmod(core, NKV)
        in_maps.append(
            {
                "xT": np.ascontiguousarray(x[b].T).astype(NPBF16),
                "wq": np.ascontiguousarray(
                    Wq4[:, :, g, :].reshape(D, QPG * DH)
                ).astype(NPBF16),
                "wkv": np.ascontiguousarray(
                    np.concatenate(
                        [
                            Wkv2[:, g * DH : (g + 1) * DH],
                            Wkv2[:, NKV * DH + g * DH : NKV * DH + (g + 1) * DH],
                        ],
                        axis=1,
                    )
                ).astype(NPBF16),
                "wout": np.ascontiguousarray(Wout4[:, g].reshape(QPG * DH, D)).astype(
                    NPBF16
                ),
                "cosw": cos.astype(NPBF16),
                "sinw": sinw.astype(NPBF16),
                "mask": mask,
            }
        )
    return in_maps


def unshard_output(results):
    out = np.zeros((B, T, D), np.float32)
    for core in range(NC):
        b = core // NKV
        out[b] += results[core]["out"]
    return out


_NC_CACHE = None


def _get_compiled():
    global _NC_CACHE
    if _NC_CACHE is None:
        _NC_CACHE = build_graph()
    return _NC_CACHE


def kernel(**inputs):
    nc = _get_compiled()
    in_maps = shard_inputs(**inputs)
    res = run_bass_kernel_spmd(nc, in_maps, core_ids=list(range(NC)))
    return unshard_output(res.results)


# revision 8
# speedup vs baseline: 1.3583x; 1.3583x over previous
"""Trainium2 Bass kernel for GQA attention with QK-RMSNorm + YaRN RoPE.

Sharding: 8 cores = 2 (batch) x 4 (KV group). Each core owns one batch
element and one KV group (4 query heads + 1 KV head). The output
projection is split along its contraction dim, so each core produces a
partial (T, D) output; the host sums the 4 group-partials per batch.

v2 highlights (vs v1 baseline, 318 us):
- All 128x128 head transposes moved off the PE onto the DMA xbar
  (dma_start(transpose=True) on the Activation HWDGE queue).
- Dual DMA queues: x tiles + outputs on Sync, weights/tables/transposes
  on Activation; everything prefetched up-front.
- Phase A projections run k-innermost over j-pairs so chunk-0 compute
  paces with DMA arrival; rope is 4 batched DVE ops (broadcast APs) +
  per-head norm-scale instead of ~20 per-head ops.
- Phase B runs a depth-3 software pipeline: scores (+exp on ACT) issue
  ahead of PV/denominator matmuls so the PE never waits on Exp; the
  out-projection (phase C) groups are interleaved as PE gap fillers.
"""

import math
import sys

import numpy as np

if "/opt/trn_rl_repo" not in sys.path:
    sys.path.insert(0, "/opt/trn_rl_repo")

import ml_dtypes

import concourse.bass as bass
import concourse.tile as tile
from concourse import bacc, mybir
from concourse.bass import broadcast_tensor_aps
from concourse.bass_utils import run_bass_kernel_spmd

# Problem constants (hardcoded; kernel.py must be self-contained).
B, T, D = 2, 2048, 2048
DH, NQ, NKV = 128, 16, 4
QPG = NQ // NKV  # 4 query heads per KV group
ROPE_BASE = 10000.0
YARN_SCALE = 2.0
ORIG_MAX_LEN = 4096
BETA_FAST, BETA_SLOW = 32.0, 1.0
EPS = 1.1920929e-07
MSCALE = 0.1 * math.log(YARN_SCALE) + 1.0
ATTN_SCALE = 1.0 / (MSCALE * math.sqrt(DH))

NC = 8  # cores
TC = 512  # tokens per chunk
NCHUNK = T // TC  # 4
NST = T // 128  # 16 s-tiles (128 tokens each)
NH = QPG + 1  # 4 q heads + 1 k head handled together in phase A

F32 = mybir.dt.float32
BF16 = mybir.dt.bfloat16
NPBF16 = ml_dtypes.bfloat16


def _yarn_inv_freq():
    inv_freq = 1.0 / ROPE_BASE ** (np.arange(0, DH, 2, dtype=np.float32) / DH)
    wavelengths = 2.0 * math.pi / inv_freq
    low_w = ORIG_MAX_LEN / BETA_SLOW
    high_w = ORIG_MAX_LEN / BETA_FAST
    gamma = np.clip((low_w - wavelengths) / (low_w - high_w), 0.0, 1.0)
    return (gamma * inv_freq + (1.0 - gamma) * inv_freq / YARN_SCALE).astype(np.float32)


def _rope_tables():
    t = np.arange(T, dtype=np.float32)
    freqs = np.outer(t, _yarn_inv_freq())  # (T, 64)
    emb = np.concatenate([freqs, freqs], axis=-1)  # (T, 128)
    cos = np.cos(emb).astype(np.float32)
    sin = np.sin(emb).astype(np.float32)
    # Signed sin table: rope term2[:, :64] = q[:, 64:] * (-sin[:, :64]),
    # term2[:, 64:] = q[:, :64] * (+sin[:, 64:]).
    sinw = sin.copy()
    sinw[:, : DH // 2] *= -1.0
    return cos, sinw


def build_graph():
    nc = bacc.Bacc("TRN2", target_bir_lowering=False, debug=False, num_devices=NC)

    xT_d = nc.dram_tensor("xT", [D, T], BF16, kind="ExternalInput").ap()
    wq_d = nc.dram_tensor("wq", [D, QPG * DH], BF16, kind="ExternalInput").ap()
    wkv_d = nc.dram_tensor("wkv", [D, 2 * DH], BF16, kind="ExternalInput").ap()
    wout_d = nc.dram_tensor("wout", [QPG * DH, D], BF16, kind="ExternalInput").ap()
    cos_d = nc.dram_tensor("cosw", [T, DH], BF16, kind="ExternalInput").ap()
    sin_d = nc.dram_tensor("sinw", [T, DH], BF16, kind="ExternalInput").ap()
    mask_d = nc.dram_tensor("mask", [128, 128], BF16, kind="ExternalInput").ap()
    out_d = nc.dram_tensor("out", [T, D], BF16, kind="ExternalOutput").ap()

    with tile.TileContext(nc) as tc:
        with tc.tile_pool(name="persist", bufs=1) as pp:
            # Head-major transposed activations: [dh, head, t]; heads 0-3
            # are q, head 4 is k.
            qkT = pp.tile([128, NH, T], BF16)
            v_tok = pp.tile([128, NST, DH], BF16)  # token-major V
            oT = pp.tile([128, QPG, T], BF16)
            mask_sb = pp.tile([128, 128], BF16)
            ones_mat = pp.tile([128, 128], BF16)
            eps_col = pp.tile([128, 1], F32)
            wout_t = pp.tile([128, QPG, NCHUNK, 512], BF16)
            nc.sync.dma_start(mask_sb[:], mask_d[:])
            nc.vector.memset(ones_mat[:], 1.0)
            nc.vector.memset(eps_col[:], EPS)

            # ---------------- Phase A: projections + norm + rope ----------
            with (
                tc.tile_pool(name="wA", bufs=1) as wA,
                tc.tile_pool(name="xt", bufs=56) as xtp,
                tc.tile_pool(name="ropetab", bufs=8) as rtp,
                tc.tile_pool(name="psA_q", bufs=4, space="PSUM") as psq,
                tc.tile_pool(name="psA_kv", bufs=4, space="PSUM") as pskv,
                tc.tile_pool(name="qk", bufs=8) as qkp,
                tc.tile_pool(name="sq", bufs=4) as sqp,
                tc.tile_pool(name="small", bufs=12) as smp,
                tc.tile_pool(name="rope", bufs=2) as rp,
                tc.tile_pool(name="hat", bufs=6) as hp,
            ):
                wq_t = wA.tile([128, D // 128, QPG * DH], BF16)
                wkv_t = wA.tile([128, D // 128, 2 * DH], BF16)

                # x tiles on the Sync HWDGE queue (all chunks up front).
                xt = []
                for c in range(NCHUNK):
                    row = []
                    for k in range(D // 128):
                        xk = xtp.tile([128, TC], BF16, name=f"xt_{c}_{k}", tag="xt")
                        nc.sync.dma_start(
                            xk[:],
                            xT_d[128 * k : 128 * (k + 1), TC * c : TC * (c + 1)],
                        )
                        row.append(xk)
                    xt.append(row)

                # Weights + rope tables on the Activation HWDGE queue.
                for k in range(D // 128):
                    nc.scalar.dma_start(
                        wq_t[:, k, :], wq_d[128 * k : 128 * (k + 1), :]
                    )
                    nc.scalar.dma_start(
                        wkv_t[:, k, :], wkv_d[128 * k : 128 * (k + 1), :]
                    )
                nc.scalar.dma_start(
                    wout_t[:],
                    wout_d.rearrange("(h p) (c n) -> p h c n", p=128, n=512),
                )
                cos_ts, sin_ts = [], []
                for c in range(NCHUNK):
                    cos_t = rtp.tile([128, 4, DH], BF16, name=f"cos_{c}", tag="cos")
                    sin_t = rtp.tile([128, 4, DH], BF16, name=f"sin_{c}", tag="sin")
                    nc.scalar.dma_start(
                        cos_t[:],
                        cos_d[TC * c : TC * (c + 1), :].rearrange(
                            "(j p) d -> p j d", p=128
                        ),
                    )
                    nc.scalar.dma_start(
                        sin_t[:],
                        sin_d[TC * c : TC * (c + 1), :].rearrange(
                            "(j p) d -> p j d", p=128
                        ),
                    )
                    cos_ts.append(cos_t)
                    sin_ts.append(sin_t)

                for c in range(NCHUNK):
                    ssq = smp.tile([128, 4 * NH], F32, name=f"ssq_{c}", tag="ssq")
                    qk_list = []
                    for half in range(2):
                        ps_qs = [
                            psq.tile([128, 512], F32, name=f"psq_{c}_{half}_{i}", tag="psq")
                            for i in range(2)
                        ]
                        ps_ks = [
                            pskv.tile([128, 256], F32, name=f"psk_{c}_{half}_{i}", tag="psk")
                            for i in range(2)
                        ]
                        for k in range(D // 128):
                            for jj in range(2):
                                j = 2 * half + jj
                                x_sl = xt[c][k][:, 128 * j : 128 * (j + 1)]
                                nc.tensor.matmul(
                                    ps_qs[jj][:],
                                    x_sl,
                                    wq_t[:, k, :],
                                    start=(k == 0),
                                    stop=(k == D // 128 - 1),
                                )
                                nc.tensor.matmul(
                                    ps_ks[jj][:],
                                    x_sl,
                                    wkv_t[:, k, :],
                                    start=(k == 0),
                                    stop=(k == D // 128 - 1),
                                )
                        for jj in range(2):
                            j = 2 * half + jj
                            st = 4 * c + j
                            # Token-major q||k tile: heads 0..3 are q, 4 is k.
                            qk = qkp.tile([128, NH * DH], BF16, name=f"qk_{st}", tag="qk")
                            nc.vector.tensor_copy(qk[:, 0:512], ps_qs[jj][:])
                            nc.vector.tensor_copy(qk[:, 512:640], ps_ks[jj][:, 0:128])
                            nc.vector.tensor_copy(v_tok[:, st, :], ps_ks[jj][:, 128:256])
                            # RMSNorm stats on ACT: Square dumps q2, accum_out
                            # sums over dh -> ssq col per (j, head).
                            q2 = sqp.tile([128, 128], F32, name=f"q2_{st}", tag="q2")
                            for h in range(NH):
                                nc.scalar.activation(
                                    q2[:], qk[:, 128 * h : 128 * (h + 1)],
                                    mybir.ActivationFunctionType.Square,
                                    accum_out=ssq[:, NH * j + h : NH * j + h + 1],
                                )
                            qk_list.append(qk)
                    # One Ln/Exp per chunk (shared ACT table set):
                    # rsqrt(ms + eps) = exp(-0.5 * ln(ssq/128 + eps))
                    lnv = smp.tile([128, 4 * NH], F32, name=f"ln_{c}", tag="lnv")
                    scv = smp.tile([128, 4 * NH], F32, name=f"sc_{c}", tag="scv")
                    nc.scalar.activation(
                        lnv[:], ssq[:], mybir.ActivationFunctionType.Ln,
                        bias=eps_col[:], scale=1.0 / DH,
                    )
                    nc.scalar.activation(
                        scv[:], lnv[:], mybir.ActivationFunctionType.Exp,
                        bias=0.0, scale=-0.5,
                    )
                    for j in range(4):
                        st = 4 * c + j
                        qk = qk_list[j]
                        # Normalize, then rope via batched DVE ops.
                        qks = rp.tile([128, NH * DH], BF16, name=f"qks_{st}", tag="qks")
                        for h in range(NH):
                            nc.vector.tensor_scalar_mul(
                                qks[:, 128 * h : 128 * (h + 1)],
                                qk[:, 128 * h : 128 * (h + 1)],
                                scv[:, NH * j + h : NH * j + h + 1],
                            )
                        qhat = hp.tile([128, NH * DH], BF16, name=f"qh_{st}", tag="qh")
                        t1 = rp.tile([128, NH * DH], BF16, name=f"t1_{st}", tag="t1")
                        t2 = rp.tile([128, NH * DH], BF16, name=f"t2_{st}", tag="t2")
                        # t1 = qks * cos (cos broadcast across the 5 heads)
                        a0 = qks[:].rearrange("p (h d) -> p h d", h=NH)
                        a1 = cos_ts[c][:, j, :].rearrange("p (o d) -> p o d", o=1)
                        b0, b1 = broadcast_tensor_aps(a0, a1)
                        t1r = t1[:].rearrange("p (h d) -> p h d", h=NH)
                        nc.vector.tensor_mul(t1r, b0, b1)
                        # t2 halves: lower-out = upper-in * (-sin_low),
                        # upper-out = lower-in * (+sin_high)
                        q4 = qks[:].rearrange("p (h t d) -> p h t d", h=NH, t=2)
                        t24 = t2[:].rearrange("p (h t d) -> p h t d", h=NH, t=2)
                        s_lo = sin_ts[c][:, j, 0:64].rearrange("p (o u d) -> p o u d", o=1, u=1)
                        s_hi = sin_ts[c][:, j, 64:128].rearrange("p (o u d) -> p o u d", o=1, u=1)
                        c0, c1 = broadcast_tensor_aps(q4[:, :, 1:2, :], s_lo)
                        nc.vector.tensor_mul(t24[:, :, 0:1, :], c0, c1)
                        c2, c3 = broadcast_tensor_aps(q4[:, :, 0:1, :], s_hi)
                        nc.vector.tensor_mul(t24[:, :, 1:2, :], c2, c3)
                        nc.vector.tensor_add(qhat[:], t1[:], t2[:])
                        # One batched head transpose per (c, j) on the DMA
                        # xbar (Sync queue): out[d, h, t] = qhat[t, 128h+d].
                        nc.sync.dma_start(
                            qkT[:, :, 128 * st : 128 * (st + 1)], qhat[:],
                            transpose=True,
                        )

            # ---------------- Phase B + C: attention + out projection -----
            with (
                tc.tile_pool(name="psB_s", bufs=4, space="PSUM") as pss,
                tc.tile_pool(name="psB_o", bufs=2, space="PSUM") as pso,
                tc.tile_pool(name="psB_d", bufs=2, space="PSUM") as psd,
                tc.tile_pool(name="ebuf", bufs=8) as ep,
                tc.tile_pool(name="bcs", bufs=3) as bcp,
                tc.tile_pool(name="osb", bufs=4) as osp,
            ):
                units = []
                for j in range(NCHUNK):
                    for h in range(QPG):
                        for st in range(4 * (j + 1)):
                            units.append((j, h, st))

                # Plan: B units with C(j-1) groups interleaved as fillers;
                # C(3) drains at the end.
                plan = []
                base = 0
                for j in range(NCHUNK):
                    n_u = QPG * 4 * (j + 1)
                    cfills = (
                        [("c", j - 1, jj, dc) for jj in range(4) for dc in range(4)]
                        if j >= 1
                        else []
                    )
                    ci = 0
                    for k in range(n_u):
                        plan.append(("u", base + k))
                        if ci < len(cfills) and (k % (j + 1)) == j:
                            plan.append(cfills[ci])
                            ci += 1
                    for item in cfills[ci:]:
                        plan.append(item)
                    base += n_u
                for jj in range(4):
                    for dc in range(4):
                        plan.append(("c", NCHUNK - 1, jj, dc))

                def emit_scores(u):
                    j, h, st = u
                    d0 = max(0, 128 * (st - 4 * j))
                    ps_s = pss.tile([128, 512], F32, name=f"pss_{j}_{h}_{st}", tag="pss")
                    nc.tensor.matmul(
                        ps_s[:, d0:512],
                        qkT[:, QPG, 128 * st : 128 * (st + 1)],
                        qkT[:, h, TC * j + d0 : TC * (j + 1)],
                        start=True,
                        stop=True,
                    )
                    E = ep.tile([128, 512], BF16, name=f"E_{j}_{h}_{st}", tag="E")
                    nc.scalar.activation(
                        E[:, d0:512], ps_s[:, d0:512],
                        mybir.ActivationFunctionType.Exp,
                        bias=0.0, scale=ATTN_SCALE,
                    )
                    if st >= 4 * j:  # diagonal block is triangular
                        nc.vector.tensor_mul(
                            E[:, d0 : d0 + 128], E[:, d0 : d0 + 128], mask_sb[:]
                        )
                    return (E, d0)

                cur = {}

                def emit_pv(u, ed):
                    j, h, st = u
                    E, d0 = ed
                    S = 4 * (j + 1)
                    if st == 0:
                        cur["o"] = pso.tile([128, 512], F32, name=f"pso_{j}_{h}", tag="pso")
                        cur["d"] = psd.tile([128, 512], F32, name=f"psd_{j}_{h}", tag="psd")
                    nc.tensor.matmul(
                        cur["o"][:, d0:512],
                        v_tok[:, st, :],
                        E[:, d0:512],
                        start=(st == 0),
                        stop=(st == S - 1),
                    )
                    nc.tensor.matmul(
                        cur["d"][:, d0:512],
                        ones_mat[:],
                        E[:, d0:512],
                        start=(st == 0),
                        stop=(st == S - 1),
                    )
                    if st == S - 1:
                        bc = bcp.tile([128, 512], F32, name=f"bc_{j}_{h}", tag="bc")
                        nc.vector.reciprocal_approx_fast(bc[:], cur["d"][:])
                        nc.vector.tensor_mul(
                            oT[:, h, TC * j : TC * (j + 1)], cur["o"][:], bc[:]
                        )

                def emit_cgroup(j, jj, dc):
                    t0 = TC * j + 128 * jj
                    ps_c = pss.tile([128, 512], F32, name=f"psc_{j}_{jj}_{dc}", tag="pss")
                    for h in range(QPG):
                        nc.tensor.matmul(
                            ps_c[:],
                            oT[:, h, t0 : t0 + 128],
                            wout_t[:, h, dc, :],
                            start=(h == 0),
                            stop=(h == QPG - 1),
                        )
                    o_sb = osp.tile([128, 512], BF16, name=f"o_{j}_{dc}_{jj}", tag="o")
                    nc.vector.tensor_copy(o_sb[:], ps_c[:])
                    nc.sync.dma_start(
                        out_d[t0 : t0 + 128, 512 * dc : 512 * (dc + 1)], o_sb[:]
                    )

                sp = 0  # scores-emitted pointer (runs ahead of PV)
                einfo = {}
                for item in plan:
                    if item[0] == "u":
                        i = item[1]
                        while sp < min(i + 3, len(units)):
                            einfo[sp] = emit_scores(units[sp])
                            sp += 1
                        emit_pv(units[i], einfo.pop(i))
                    else:
                        emit_cgroup(item[1], item[2], item[3])

    nc.compile()
    return nc


def shard_inputs(x, Wq, Wkv, Wout, q_norm_w, k_norm_w, inv_freq):
    """Build per-core input maps. Weights/x are pre-cast to bf16 on host
    (compute dtype), halving their HBM traffic."""
    cos, sinw = _rope_tables()
    qw = np.asarray(q_norm_w, np.float32)
    kw = np.asarray(k_norm_w, np.float32)
    assert np.allclose(qw, 1.0) and np.allclose(kw, 1.0), "non-unit norm weights"

    mask = np.triu(np.ones((128, 128), np.float32)).astype(NPBF16)
    Wq4 = np.asarray(Wq, np.float32).reshape(D, QPG, NKV, DH)
    Wkv2 = np.asarray(Wkv, np.float32)
    Wout4 = np.asarray(Wout, np.float32).reshape(QPG, NKV, DH, D)
    x = np.asarray(x, np.float32)

    in_maps = []
    for core in range(NC):
        b, g = divmod(core, NKV)
        in_maps.append(
            {
                "xT": np.ascontiguousarray(x[b].T).astype(NPBF16),
                "wq": np.ascontiguousarray(
                    Wq4[:, :, g, :].reshape(D, QPG * DH)
                ).astype(NPBF16),
                "wkv": np.ascontiguousarray(
                    np.concatenate(
                        [
                            Wkv2[:, g * DH : (g + 1) * DH],
                            Wkv2[:, NKV * DH + g * DH : NKV * DH + (g + 1) * DH],
                        ],
                        axis=1,
                    )
                ).astype(NPBF16),
                "wout": np.ascontiguousarray(Wout4[:, g].reshape(QPG * DH, D)).astype(
                    NPBF16
                ),
                "cosw": cos.astype(NPBF16),
                "sinw": sinw.astype(NPBF16),
                "mask": mask,
            }
        )
    return in_maps


def unshard_output(results):
    out = np.zeros((B, T, D), np.float32)
    for core in range(NC):
        b = core // NKV
        out[b] += results[core]["out"]
    return out


_NC_CACHE = None


def _get_compiled():
    global _NC_CACHE
    if _NC_CACHE is None:
        _NC_CACHE = build_graph()
    return _NC_CACHE


def kernel(**inputs):
    nc = _get_compiled()
    in_maps = shard_inputs(**inputs)
    res = run_bass_kernel_spmd(nc, in_maps, core_ids=list(range(NC)))
    return unshard_output(res.results)
